# revision 1
# baseline (speedup 1.0000x reference)
"""Multi-head attention (B=2, S=4096, D=1024, H=16, HD=64) on 8 trn2 cores.

Sharding: core c -> batch b = c//4, head-group g = c%4 (4 heads per core).
Each core: Q/K/V projections for its heads on its batch, attention, and the
partial output ctx @ Wo[rows of its heads]. Host sums the 4 partials per
batch and adds bo.

Layout choices (vs the fp32r baseline, measured on HW):
  - All matmul operands bf16 (host pre-converts X/W; fp32 streams at ~half
    rate on the PE, bf16 at 1 col/cycle). PSUM accumulation stays fp32.
  - X arrives pre-TRANSPOSED AND pre-TILED from the host
    (XT [t8, dc, 128, 512] bf16): no on-chip transposition; every xt tile
    is one contiguous 128 KB DMA. Weight DMAs are split per 64 KB slice and
    staged behind the first X chunk so the first projection matmul starts
    as early as possible.
  - exp split ~50:50 across two engines: scalar (ACT, exact exp, bf16 out,
    1113ns per [128,1024]) and vector (Schraudolph bit-trick:
    int16(x*a+b) bitcast to bf16, ~3.5% max err, 1223ns). Together they
    keep up with the PE's ~870ns/triple. Boundary triples (k 0/31) go to
    ACT so the DVE reaches the ctx-evict copies quickly.
  - PV is a single full-K (128-token) matmul per head - in bf16 the
    fp32-era row-split pair has no advantage, and one acc per head frees
    2 PSUM banks, buying a third st buffer (LOOKAHEAD=3) to hide exp
    latency. (NB: two row-tiled matmuls accumulating into the SAME bank
    crash the device - PSUM banks cannot be shared across row tiles.)
  - ctx'' [65, S] per head stays resident in SBUF bf16 (no DRAM spill).
  - Phase 3 normalizes via transpose -> per-partition reciprocal multiply
    -> transpose back, software-pipelined two stages deep across token
    tiles to hide the cross-engine chain latency.
  - Output partial written fp16 (half the DMA of fp32, plenty of mantissa).
  - PE warm-up spin at t=0 so the HAM clock-gate opens before real work.
"""

import os
from contextlib import ExitStack

import numpy as np
import ml_dtypes

os.environ.setdefault("MYCRO_LOCAL_CACHE", "1")

import concourse.bass as bass
import concourse.tile as tile
from concourse import bacc, mybir
from concourse.bass_utils import run_bass_kernel_spmd
from concourse.masks import make_identity

F32 = mybir.dt.float32
BF16 = mybir.dt.bfloat16
FP16 = mybir.dt.float16
I16 = mybir.dt.int16
AF = mybir.ActivationFunctionType
OP = mybir.AluOpType

S = 4096          # sequence length
D = 1024          # model dim
HC = 4            # heads per core
HD = 64           # head dim
DC = HC * HD      # 256 per-core projection width
NP = HC // 2      # head pairs per core
KT = S // 128     # 32 k-tiles
QC = S // 512     # 8 q-chunks of 512
SCALE = 1.0 / 8.0

LN2 = float(np.log(2.0))
EXP_A = (128.0 / LN2) * SCALE          # Schraudolph slope (scale folded in)
EXP_B = 127.0 * 128.0 - 5.84           # Schraudolph offset (bf16 flavor)


def _emit(ctx: ExitStack, tc: tile.TileContext, ins: dict, out: bass.AP):
    nc = tc.nc
    X, Wq, bq, Wk, bk, Wv, bv, Wo = (
        ins["XT"], ins["Wq"], ins["bq"], ins["Wk"], ins["bk"], ins["Wv"],
        ins["bv"], ins["Wo"],
    )

    const = ctx.enter_context(tc.tile_pool(name="const", bufs=1))
    ident = const.tile([128, 128], BF16)
    make_identity(nc, ident[:])

    # PE warm-up: matmuls with no DMA dependency so the HAM clock-gate
    # opens (~3.4us of activity) while the weight DMAs are in flight.
    with tc.tile_pool(name="warm", bufs=2, space="PSUM") as warm:
        wt = [warm.tile([128, 128], F32, tag=f"w{i}", name=f"warm{i}")
              for i in range(2)]
        for i in range(38):
            nc.tensor.matmul(wt[i % 2][:], ident[:], ident[:],
                             start=True, stop=True)

    # Weights arrive pre-converted to bf16 from the host - direct DMA.
    # (Issued interleaved with the X chunk DMAs inside phase 1 so the first
    # projection matmuls start as early as possible.)
    wq_sb = const.tile([128, 8 * DC], BF16, tag="wq")
    wk_sb = const.tile([128, 8 * DC], BF16, tag="wk")
    wv_sb = const.tile([128, 8 * DC], BF16, tag="wv")
    wo_sb = const.tile([128, 2 * D], BF16, tag="wo")
    bq_sb = const.tile([128, 2], F32, tag="bq")
    bk_sb = const.tile([128, 2], F32, tag="bk")
    nc.sync.dma_start(bq_sb[:], bq.rearrange("(c p) -> p c", p=128))
    nc.scalar.dma_start(bk_sb[:], bk.rearrange("(c p) -> p c", p=128))
    bv_bc = const.tile([128, DC], F32, tag="bv")
    nc.sync.dma_start(bv_bc[:], bv.unsqueeze(0).to_broadcast([128, DC]))

    def _wdma(eng, dst, src, nchunks):
        # per-chunk transfers so the first projection matmul only waits for
        # the 64 KB slice it actually reads
        sv = src.rearrange("(c p) d -> c p d", p=128)
        for c in range(nchunks):
            w = dst.shape[1] // nchunks
            eng.dma_start(dst[:, c * w:(c + 1) * w], sv[c])

    # ctx'' [65, S] per head lives in SBUF through phases 2-3.
    ctxp = ctx.enter_context(tc.tile_pool(name="ctxp", bufs=1))
    CTX = [ctxp.tile([65, S], BF16, tag=f"ctx{h}", name=f"ctx{h}")
           for h in range(HC)]

    # Activations for phases 1-2 (freed before phase 3).
    acts_ctx = ExitStack()
    acts = acts_ctx.enter_context(tc.tile_pool(name="acts", bufs=1))
    QT = [acts.tile([128, S], BF16, tag=f"qt{p}", name=f"qt{p}") for p in range(NP)]
    KT_ = [acts.tile([128, S], BF16, tag=f"kt{p}", name=f"ktile{p}") for p in range(NP)]
    VPA = acts.tile([128, KT, HC * 65], BF16, tag="vpa", name="vpa")
    VP = [VPA[:, k, :] for k in range(KT)]
    # ones columns for the in-matmul softmax denominator, set once
    nc.vector.memset(
        VPA[:].rearrange("p k (h w) -> p k h w", h=HC)[:, :, :, 64:65], 1.0)

    # ---------------- Phase 1: DMA X^T + projections ----------
    # X arrives host-tiled as [t8, dc, 128, 512] so each xt tile is one
    # fully contiguous 128 KB DMA.
    with nc.named_scope("ph1"), \
         tc.tile_pool(name="xt", bufs=32) as xtp, \
         tc.tile_pool(name="ps2", bufs=3, space="PSUM") as ps2:
        wq_v = Wq.rearrange("(c p) d -> c p d", p=128)
        wk_v = Wk.rearrange("(c p) d -> c p d", p=128)
        for t8 in range(8):
            ts8 = slice(t8 * 512, (t8 + 1) * 512)
            xt = [xtp.tile([128, 512], BF16, tag="xt", name=f"xt{t8}_{i}")
                  for i in range(8)]
            for dc in range(8):
                eng = nc.sync if dc % 2 == 0 else nc.scalar
                if t8 == 0:
                    # interleave the wq/wk slice each projection matmul
                    # needs right before its xt tile, so the dc-chain
                    # streams as the transfers land
                    eng.dma_start(wq_sb[:, dc * DC:(dc + 1) * DC], wq_v[dc])
                    eng.dma_start(wk_sb[:, dc * DC:(dc + 1) * DC], wk_v[dc])
                eng.dma_start(xt[dc][:], X[t8, dc])
            if t8 == 0:
                _wdma(nc.sync, wv_sb, Wv, 8)
            elif t8 == 1:
                _wdma(nc.scalar, wo_sb, Wo, 2)
            for p in range(NP):
                pq = ps2.tile([128, 512], F32, tag="pq")
                for dc in range(8):
                    nc.tensor.matmul(
                        pq[:], wq_sb[:, dc * DC + p * 128: dc * DC + (p + 1) * 128],
                        xt[dc][:], start=(dc == 0), stop=(dc == 7))
                nc.vector.tensor_scalar_add(
                    QT[p][:, ts8], pq[:], bq_sb[:, p:p + 1])
                pk = ps2.tile([128, 512], F32, tag="pq")
                for dc in range(8):
                    nc.tensor.matmul(
                        pk[:], wk_sb[:, dc * DC + p * 128: dc * DC + (p + 1) * 128],
                        xt[dc][:], start=(dc == 0), stop=(dc == 7))
                nc.vector.tensor_scalar_add(
                    KT_[p][:, ts8], pk[:], bk_sb[:, p:p + 1])
            for tt in range(4):
                kt = t8 * 4 + tt
                pv = ps2.tile([128, 256], F32, tag="pv")
                for dc in range(8):
                    nc.tensor.matmul(
                        pv[:], xt[dc][:, tt * 128:(tt + 1) * 128],
                        wv_sb[:, dc * DC:(dc + 1) * DC],
                        start=(dc == 0), stop=(dc == 7))
                vdst = VP[kt][:].rearrange("p (h w) -> p h w", h=HC)[:, :, 0:64]
                nc.vector.scalar_tensor_tensor(
                    vdst, pv[:].rearrange("p (h w) -> p h w", h=HC), 1.0,
                    bv_bc[:].rearrange("p (h w) -> p h w", h=HC),
                    OP.bypass, OP.add)

    # ---------------- Phase 2: attention ----------------
    with nc.named_scope("ph2"), \
         tc.tile_pool(name="sps", bufs=3, space="PSUM") as sps, \
         tc.tile_pool(name="pvs", bufs=2, space="PSUM") as pvs, \
         tc.tile_pool(name="et", bufs=8) as etp:
        seq = [(p, qc, k) for p in range(NP) for qc in range(QC)
               for k in range(KT)]
        accs = {}
        ets = {}

        def s_step(i):
            p, qc, k = seq[i]
            qs = slice(qc * 512, (qc + 1) * 512)
            ks = slice(k * 128, (k + 1) * 128)
            st = sps.tile([128, 1024], F32, tag="st", name=f"st{p}_{qc}_{k}")
            nc.tensor.matmul(st[:, 0:512], KT_[p][0:64, ks],
                             QT[p][0:64, qs], start=True, stop=True)
            nc.tensor.matmul(st[:, 512:1024], KT_[p][64:128, ks],
                             QT[p][64:128, qs], start=True, stop=True)
            et = etp.tile([128, 1024], BF16, tag="et", name=f"et{p}_{qc}_{k}")
            # boundary triples (k 0/31) go to ACT so the DVE queue reaches
            # the ctx-evict copies quickly at (p,qc) transitions
            if k in (0, KT - 1) or i % 2 == 0:
                nc.scalar.activation(et[:], st[:], AF.Exp, bias=0.0,
                                     scale=SCALE)
            else:
                nc.vector.tensor_scalar(et[:].bitcast(I16), st[:],
                                        EXP_A, EXP_B, OP.mult, OP.add)
            ets[i] = et

        LOOKAHEAD = 3
        for i in range(LOOKAHEAD):
            s_step(i)
        for i, (p, qc, k) in enumerate(seq):
            if k == 0:
                accs[(p, qc)] = [
                    pvs.tile([65, 512], F32, tag="acc", name=f"acc{p}_{qc}_{j2}")
                    for j2 in range(2)]
            acc = accs[(p, qc)]
            et = ets.pop(i)
            # issue the lookahead scores before PV for max et slack, except
            # at combine boundaries where the ctx-evict copies must reach
            # the DVE queue before the next exp
            if k != KT - 1 and i + LOOKAHEAD < len(seq):
                s_step(i + LOOKAHEAD)
            for j in range(2):
                h = 2 * p + j
                vs = slice(h * 65, (h + 1) * 65)
                es = slice(j * 512, (j + 1) * 512)
                # single full-K (128-token) matmul per head: bf16 streams
                # 1 col/cycle either way, so no need for the fp32-era
                # row-split pair
                nc.tensor.matmul(
                    acc[j][:], VP[k][:, vs], et[:, es],
                    start=(k == 0), stop=(k == KT - 1),
                    skip_group_check=True)
            if k == KT - 1:
                qs = slice(qc * 512, (qc + 1) * 512)
                for j in range(2):
                    nc.vector.tensor_copy(CTX[2 * p + j][:, qs], acc[j][:])
                del accs[(p, qc)]
                if i + LOOKAHEAD < len(seq):
                    s_step(i + LOOKAHEAD)

    acts_ctx.close()

    # ---------------- Phase 3: normalize + Wo ----------------
    with nc.named_scope("ph3"), \
         tc.tile_pool(name="ps3a", bufs=3, space="PSUM") as ps3a, \
         tc.tile_pool(name="ps3b", bufs=2, space="PSUM") as ps3b, \
         tc.tile_pool(name="po", bufs=3, space="PSUM") as pop, \
         tc.tile_pool(name="ctxn", bufs=3) as ctxnp, \
         tc.tile_pool(name="rcpp", bufs=3) as rcpp, \
         tc.tile_pool(name="ltp", bufs=3) as ltp, \
         tc.tile_pool(name="osb", bufs=3) as osbp:
        ctxns = {}
        lts = {}

        def t_head(t):
            ts_ = slice(t * 128, (t + 1) * 128)
            # all 4 heads' ctx columns transposed into one psum tile
            # (66-wide slots so the denominator column stays 4B-aligned)
            tp1 = ps3a.tile([128, HC * 66], BF16, tag="tp1", name=f"tp1_{t}")
            for h in range(HC):
                nc.tensor.transpose(tp1[:, h * 66:h * 66 + 65],
                                    CTX[h][:, ts_], ident[0:65, 0:65])
            tp1v = tp1[:].rearrange("p (h w) -> p h w", h=HC)
            rcp = rcpp.tile([128, HC], F32, tag="rcp", name=f"rcp{t}")
            nc.vector.reciprocal(rcp[:].unsqueeze(2), tp1v[:, :, 64:65])
            ctxn = ctxnp.tile([128, 2 * 128], BF16, tag="ctxn",
                              name=f"ctxn{t}")
            nc.vector.tensor_tensor(
                ctxn[:].rearrange("p (h w) -> p h w", h=HC),
                tp1v[:, :, 0:64],
                rcp[:].unsqueeze(2).to_broadcast([128, HC, 64]),
                mybir.AluOpType.mult)
            ctxns[t] = ctxn

        def t_mid(t):
            ctxn = ctxns.pop(t)
            tp2 = ps3b.tile([128, 2 * 128], BF16, tag="tp2", name=f"tp2_{t}")
            for p in range(NP):
                nc.tensor.transpose(tp2[:, p * 128:(p + 1) * 128],
                                    ctxn[:, p * 128:(p + 1) * 128], ident[:])
            lt = ltp.tile([128, 2 * 128], BF16, tag="lt", name=f"lt{t}")
            nc.scalar.copy(lt[:], tp2[:])
            lts[t] = lt

        def t_tail(t):
            ts_ = slice(t * 128, (t + 1) * 128)
            lt = lts.pop(t)
            ot = osbp.tile([128, D], FP16, tag="ot", name=f"ot{t}")
            for n2 in range(2):
                po = pop.tile([128, 512], F32, tag="po", name=f"po{t}_{n2}")
                for p in range(NP):
                    nc.tensor.matmul(
                        po[:], lt[:, p * 128:(p + 1) * 128],
                        wo_sb[:, p * D + n2 * 512: p * D + (n2 + 1) * 512],
                        start=(p == 0), stop=(p == NP - 1))
                if n2 == 0:
                    nc.scalar.copy(ot[:, 0:512], po[:])
                else:
                    nc.vector.tensor_copy(ot[:, 512:1024], po[:])
            # sync queue only: a dma_start costs ~590ns of issuing-engine
            # time, and the scalar engine is phase 3's critical path
            nc.sync.dma_start(out[ts_, :], ot[:])

        # software-pipelined two stages deep so the cross-engine normalize
        # chain of tile t overlaps the transposes/matmuls of its neighbors
        NT = S // 128
        t_head(0)
        t_head(1)
        t_mid(0)
        for t in range(NT):
            if t + 2 < NT:
                t_head(t + 2)
            if t + 1 < NT:
                t_mid(t + 1)
            t_tail(t)

_CACHE = {}


def _build():
    if "nc" in _CACHE:
        return _CACHE["nc"]
    nc = bacc.Bacc("TRN2", target_bir_lowering=False, debug=False)
    ins = {
        "XT": nc.dram_tensor("XT", [8, 8, 128, 512], BF16,
                             kind="ExternalInput").ap(),
        "Wq": nc.dram_tensor("Wq", [D, DC], BF16, kind="ExternalInput").ap(),
        "bq": nc.dram_tensor("bq", [DC], F32, kind="ExternalInput").ap(),
        "Wk": nc.dram_tensor("Wk", [D, DC], BF16, kind="ExternalInput").ap(),
        "bk": nc.dram_tensor("bk", [DC], F32, kind="ExternalInput").ap(),
        "Wv": nc.dram_tensor("Wv", [D, DC], BF16, kind="ExternalInput").ap(),
        "bv": nc.dram_tensor("bv", [DC], F32, kind="ExternalInput").ap(),
        "Wo": nc.dram_tensor("Wo", [DC, D], BF16, kind="ExternalInput").ap(),
    }
    outp = nc.dram_tensor("out", [S, D], FP16, kind="ExternalOutput").ap()
    with tile.TileContext(nc) as tcx:
        with ExitStack() as ctx:
            _emit(ctx, tcx, ins, outp)
    nc.compile()
    _CACHE["nc"] = nc
    return nc


def core_inputs(X, Wq, bq, Wk, bk, Wv, bv, Wo, core):
    b, g = divmod(core, 4)
    cs = slice(g * DC, (g + 1) * DC)
    bf = ml_dtypes.bfloat16
    xt_tiled = np.ascontiguousarray(
        X[b].T.astype(bf).reshape(8, 128, 8, 512).transpose(2, 0, 1, 3))
    return {
        "XT": xt_tiled,
        "Wq": np.ascontiguousarray(Wq[:, cs]).astype(bf),
        "bq": np.ascontiguousarray(bq[cs]),
        "Wk": np.ascontiguousarray(Wk[:, cs]).astype(bf),
        "bk": np.ascontiguousarray(bk[cs]),
        "Wv": np.ascontiguousarray(Wv[:, cs]).astype(bf),
        "bv": np.ascontiguousarray(bv[cs]),
        "Wo": np.ascontiguousarray(Wo[cs, :]).astype(bf),
    }


def kernel(X, Wq, bq, Wk, bk, Wv, bv, Wo, bo, _trace=False):
    nc = _build()
    in_maps = [core_inputs(X, Wq, bq, Wk, bk, Wv, bv, Wo, c) for c in range(8)]
    res = run_bass_kernel_spmd(nc, in_maps, list(range(8)), trace=_trace)
    parts = [res.results[c]["out"].astype(np.float32) for c in range(8)]
    full = np.stack([
        parts[0] + parts[1] + parts[2] + parts[3] + bo,
        parts[4] + parts[5] + parts[6] + parts[7] + bo,
    ]).astype(np.float32)
    if _trace:
        return full, res
    return full

